# revision 41
# baseline (speedup 1.0000x reference)
"""GAT attention kernel for Trainium2 (Bass/Tile), 8-core data parallel.

Per-core math (2 examples each, N=256 items, D=64):
  e   = LayerNorm(emb);  ua = e[0] * e[2:]
  qk  = LeakyReLU(s_q_i + s_k_j + c);  alpha = softmax_j
  attention over value_ij = LN(ua_i * ua_j) collapsed via gram matrices:
    mu = UA@UA^T/D,  E2 = UA^2@UA^2^T/D,  var = E2/D - (mu/D)^2
    rstd = exp(-0.5*ln(var + eps))   <- ln+exp live in ONE act-table set,
                                        so the kernel needs a single
                                        ACT_TABLE_LOAD (no Abs_rsqrt set)
    att_i = g*((ua_i*St_i - ct_i) * rden_i) + b
  with St = beta~@UA, beta~ = exp(qk)*rstd (unnormalized), rden = 1/sum_j exp,
  ct_i = rowsum(ua_i*St_i)/D.
  out = LeakyReLU(concat([e0*e1], att))

Perf structure vs the previous version:
  - no fp32->fp32r cast copies: fp32r tiles are written through .bitcast(F32)
    views (bit-identical), so the Scalar queue has no preamble copy chain.
  - LN statistics: square+sum fused into one DVE scalar_tensor_tensor with
    accum_out (no ACT round trip), all 5 inv-sigma values (U rows + 4 item
    blocks) batched into ONE quake-rsqrt chain.
  - scores: s_i computed with a transpose of eln(U) + a [1,128] matmul
    (kills both SBUF->SBUF iid DMAs); u0*iid output row computed in place
    at partitions {32, 96} and stored with one 2-descriptor DMA.
  - variance path: mu^2 on ACT (Square, scale), E2/D - msq fused in one DVE
    stt straight out of PSUM; rstd = exp(-0.5 ln(var+eps)) on ACT.
  - all j,i-transposed beta~ logic as before: no PE transposes of beta.
  - inputs arrive in 3 DMAs spread over 3 queues, outputs in 3 DMAs.
"""

import numpy as np

import concourse.bass as bass
from concourse import bacc
import concourse.mybir as mybir
import concourse.tile as tile
from concourse import masks
from concourse.bass_utils import run_bass_kernel_spmd
from concourse.tile import add_dep_helper

F32 = mybir.dt.float32
F32R = mybir.dt.float32r
I32 = mybir.dt.int32
ALU = mybir.AluOpType
ACTF = mybir.ActivationFunctionType
AX = mybir.AxisListType

B, NODE, D = 16, 258, 64
N = NODE - 2
N_CORES = 8
B_LOC = B // N_CORES
EPS = 1e-5
SLOPE = 0.01
OUT_ROWS = N + 1
MAGIC = 0x5f375a86


def _rsqrt(nc, pool, x, P, W, pfx):
    """x**-0.5 on DVE: bit trick + 1 Newton iteration. rel err ~1.8e-3."""
    y0 = pool.tile([P, W], F32, tag=pfx + "_y0")
    nc.vector.tensor_scalar(y0.bitcast(I32)[:], x.bitcast(I32)[:], 1, None,
                            op0=ALU.logical_shift_right)
    nc.vector.tensor_scalar(y0.bitcast(I32)[:], y0.bitcast(I32)[:], -1, MAGIC,
                            op0=ALU.mult, op1=ALU.add)
    t = pool.tile([P, W], F32, tag=pfx + "_t")
    nc.vector.tensor_mul(t[:], y0[:], y0[:])
    u = pool.tile([P, W], F32, tag=pfx + "_u")
    nc.vector.scalar_tensor_tensor(u[:], t[:], 0.5, x[:], op0=ALU.mult, op1=ALU.mult)
    v = pool.tile([P, W], F32, tag=pfx + "_v")
    nc.vector.tensor_mul(v[:], u[:], y0[:])
    r = pool.tile([P, W], F32, tag=pfx + "_r")
    nc.vector.scalar_tensor_tensor(r[:], y0[:], 1.5, v[:], op0=ALU.mult, op1=ALU.subtract)
    return r


def _lrelu(nc, out_ap, in_ap):
    nc.vector.scalar_tensor_tensor(out_ap, in_ap, SLOPE, in_ap, op0=ALU.mult, op1=ALU.max)


def build():
    nc = bacc.Bacc()
    emb = nc.dram_tensor("emb", [B_LOC, NODE, D], F32, kind="ExternalInput")
    cstT = nc.dram_tensor("cstT", [D, 3], F32R, kind="ExternalInput")   # cols: vq, vk, vi
    cstR = nc.dram_tensor("cstR", [1, 4 * D], F32R, kind="ExternalInput")  # [g|b|c0|pad]
    out = nc.dram_tensor("out", [B_LOC, OUT_ROWS, D], F32, kind="ExternalOutput")

    with tile.TileContext(nc) as tc:
        with (
            tc.tile_pool(name="const", bufs=1) as cpool,
            tc.tile_pool(name="work", bufs=2) as pool,
            tc.tile_pool(name="psmall", bufs=2, space="PSUM") as psmall,
            tc.tile_pool(name="pt", bufs=1, space="PSUM") as ppt,
            tc.tile_pool(name="pqk", bufs=1, space="PSUM") as pqk,
            tc.tile_pool(name="pmue2", bufs=2, space="PSUM") as pmue2,
            tc.tile_pool(name="ps", bufs=2, space="PSUM") as ps,
        ):
            # ---- input DMAs first on each queue: they gate the pipeline ----
            # item rows 2..257 as [128, 2, 64], row r = 2p + n
            tAB0 = pool.tile([128, 2, D], F32, tag="tAB0")
            nc.sync.dma_start(tAB0[:], emb[0, 2:258, :].rearrange("(p n) d -> p n d", n=2))
            # U+iid rows at quadrant partitions {0,32,64,96} (engine ops need
            # base partitions at quadrant boundaries)
            tU4 = cpool.tile([128, D], F32)
            u4v = tU4[:].rearrange("(a b) d -> a b d", b=32)
            nc.sync.dma_start(u4v[0:2, 0:1, :], emb[0, 0:2, :])
            nc.gpsimd.dma_start(u4v[2:4, 0:1, :], emb[1, 0:2, :])
            tAB1 = pool.tile([128, 2, D], F32, tag="tAB1")
            nc.scalar.dma_start(tAB1[:], emb[1, 2:258, :].rearrange("(p n) d -> p n d", n=2))
            in_tiles = [tAB0, tAB1]

            cst_sb = cpool.tile([1, 4 * D], F32R)
            nc.gpsimd.dma_start(cst_sb[:], cstR[:, :])
            c0_ap = cst_sb.bitcast(F32)[:, 2 * D:2 * D + 1]
            vqk = cpool.tile([D, 3], F32R)
            nc.gpsimd.dma_start(vqk[:], cstT[:, :])

            # ---- constants (casts on DVE so the ACT queue stays clean) ----
            ones2f = cpool.tile([128, 128], F32)
            nc.gpsimd.memset(ones2f[:], 1.0)
            ones2 = cpool.tile([128, 128], F32R)
            nc.vector.tensor_copy(ones2[:], ones2f[:])
            identF = cpool.tile([128, 128], F32)
            masks.make_identity(nc, identF[:])
            identR = cpool.tile([128, 128], F32R)
            nc.vector.tensor_copy(identR[:], identF[:])

            p_gb = psmall.tile([128, 2 * D], F32, tag="small")
            nc.tensor.matmul(p_gb[:], ones2[0:1, :], cst_sb[:, 0:2 * D])
            gb_bc = cpool.tile([128, 2 * D], F32)
            nc.vector.tensor_copy(gb_bc[:], p_gb[:])
            g_bc = gb_bc[:, 0:D]
            b_bc = gb_bc[:, D:2 * D]

            # ---- LN statistics, all on DVE (fused square+accum) ----
            sum5 = pool.tile([128, 5], F32, tag="sum5")
            nc.vector.reduce_sum(sum5[:, 0:1], tU4[:], axis=AX.X)
            for e in range(B_LOC):
                nc.vector.reduce_sum(sum5[:, 1 + 2 * e:3 + 2 * e], in_tiles[e][:], axis=AX.X)
            nm5 = pool.tile([128, 5], F32, tag="nm5")
            nc.vector.tensor_scalar_mul(nm5[:], sum5[:], -1.0 / D)

            ss5 = pool.tile([128, 5], F32, tag="ss5")
            xcU = pool.tile([128, D], F32, tag="xcU")
            nc.vector.tensor_scalar_add(xcU[:], tU4[:], nm5[:, 0:1])
            sq_scr = pool.tile([128, D], F32, tag="sq_scr")
            nc.vector.scalar_tensor_tensor(sq_scr[:], xcU[:], 1.0, xcU[:],
                                           op0=ALU.mult, op1=ALU.mult,
                                           accum_out=ss5[:, 0:1])
            xcs = []
            for e in range(B_LOC):
                for n in range(2):
                    xc = pool.tile([128, D], F32, tag=f"xc{e}{n}")
                    nc.vector.tensor_scalar_add(xc[:], in_tiles[e][:, n, :],
                                                nm5[:, 1 + 2 * e + n:2 + 2 * e + n])
                    scr = pool.tile([128, D], F32, tag="sq_scr2")
                    nc.vector.scalar_tensor_tensor(scr[:], xc[:], 1.0, xc[:],
                                                   op0=ALU.mult, op1=ALU.mult,
                                                   accum_out=ss5[:, 1 + 2 * e + n:2 + 2 * e + n])
                    xcs.append(xc)
            var5 = pool.tile([128, 5], F32, tag="var5")
            nc.vector.tensor_scalar(var5[:], ss5[:], 1.0 / D, EPS, op0=ALU.mult, op1=ALU.add)
            rstd5 = _rsqrt(nc, pool, var5, 128, 5, "ln5")

            # eln of U rows (all 4 quadrant rows batched)
            elnU4 = cpool.tile([128, D], F32R)
            nc.vector.scalar_tensor_tensor(elnU4[:], xcU[:], rstd5[:, 0:1],
                                           g_bc, op0=ALU.mult, op1=ALU.mult)
            nc.vector.tensor_add(elnU4[:], elnU4.bitcast(F32)[:], b_bc)

            # s_i = vi . eln(iid) for both examples via one transpose + matmul
            p_eT = psmall.tile([D, 128], F32R, tag="small")
            nc.tensor.transpose(p_eT[:], elnU4[:], identR[:])
            eT = pool.tile([D, 128], F32R, tag="eT")
            nc.vector.tensor_copy(eT[:], p_eT.bitcast(F32)[:])
            p_si = psmall.tile([1, 128], F32, tag="small")
            nc.tensor.matmul(p_si[:], vqk[:, 2:3], eT[:])
            c_all = pool.tile([1, 2], F32, tag="c_all")
            for e in range(B_LOC):
                nc.vector.tensor_scalar_add(c_all[:, e:e + 1],
                                            p_si[0:1, 32 + 64 * e:33 + 64 * e], c0_ap)

            st = [dict() for _ in range(B_LOC)]

            # ================= pass A: per-example ua + transposes =========
            uo2 = cpool.tile([128, D], F32)
            for e in range(B_LOC):
                S = st[e]
                # broadcast u0 row (same base partition trick: ones row at 64e)
                p_u0 = ps.tile([128, D], F32, tag="S")
                nc.tensor.matmul(p_u0[:], ones2[64 * e:64 * e + 1, 0:128],
                                 elnU4[64 * e:64 * e + 1, :])

                elnA = pool.tile([128, D], F32, tag="elnA")
                nc.vector.scalar_tensor_tensor(elnA[:], xcs[2 * e][:], rstd5[:, 1 + 2 * e:2 + 2 * e],
                                               g_bc, op0=ALU.mult, op1=ALU.mult)
                nc.vector.tensor_add(elnA[:], elnA[:], b_bc)
                elnB = pool.tile([128, D], F32, tag="elnB")
                nc.vector.scalar_tensor_tensor(elnB[:], xcs[2 * e + 1][:], rstd5[:, 2 + 2 * e:3 + 2 * e],
                                               g_bc, op0=ALU.mult, op1=ALU.mult)
                nc.vector.tensor_add(elnB[:], elnB[:], b_bc)
                ua_both = pool.tile([128, 2, D], F32R, tag="ua_both")
                nc.vector.tensor_mul(ua_both[:, 0, :], elnA[:], p_u0[:])
                nc.vector.tensor_mul(ua_both[:, 1, :], elnB[:], p_u0[:])
                S["ua_both"] = ua_both
                S["p_u0"] = p_u0

                # UA^T and (UA^2)^T
                p_t = ppt.tile([D, N], F32R, tag="pt")
                nc.tensor.transpose(p_t[:, 0:128], ua_both[:, 0, :], identR[:])
                nc.tensor.transpose(p_t[:, 128:256], ua_both[:, 1, :], identR[:])
                uat = pool.tile([D, N], F32R, tag="uat")
                nc.vector.tensor_copy(uat[:], p_t.bitcast(F32)[:])
                ua2t = pool.tile([D, N], F32R, tag="ua2t")
                nc.vector.tensor_mul(ua2t[:], p_t.bitcast(F32)[:],
                                     uat.bitcast(F32)[:])
                S["uat"] = uat
                S["ua2t"] = ua2t

            # ============ pass A2a: scores + exp (ACT stream) ==============
            exp_insts = []
            for e in range(B_LOC):
                S = st[e]
                uat = S["uat"]

                # s_k columns per j-block
                sk_sb = pool.tile([128, 2], F32, tag="sk_sb")
                for J in range(2):
                    cs = slice(J * 128, (J + 1) * 128)
                    p_sqk = psmall.tile([128, 2], F32, tag="small")
                    nc.tensor.matmul(p_sqk[:], uat[:, cs], vqk[:, 0:2])
                    nc.vector.tensor_copy(sk_sb[:, J:J + 1], p_sqk[:, 1:2])

                # s_q as a row, + (s_i + c0)
                p_sqrow = psmall.tile([1, N], F32, tag="small")
                nc.tensor.matmul(p_sqrow[:], vqk[:, 0:1], uat[:])
                sqc = pool.tile([1, N], F32R, tag="sqc")
                nc.scalar.activation(sqc[:], p_sqrow[:], ACTF.Identity,
                                     bias=c_all[:, e:e + 1])

                # qk^T = Prelu(bcast(s_q row) + s_k col bias); exp (unnormalized)
                p_qkT = pqk.tile([128, N], F32, tag="qk")
                nc.tensor.matmul(p_qkT[:], ones2[0:1, :], sqc[:])
                expv = pool.tile([128, 2, N], F32R, tag=f"expv{e}")
                for J in range(2):
                    qkT = pool.tile([128, N], F32, tag="qkT")
                    nc.scalar.activation(qkT[:], p_qkT[:], ACTF.Prelu,
                                         bias=sk_sb[:, J:J + 1], alpha=SLOPE)
                    ei = nc.scalar.activation(expv[:, J, :], qkT[:], ACTF.Exp)
                    exp_insts.append(ei)
                S["expv"] = expv

            # u0*iid output rows, in place at partitions {32, 96}; emitted
            # this late so the tiny ops sit behind the critical DVE work
            for e in range(B_LOC):
                r = 32 + 64 * e
                nc.vector.tensor_mul(uo2[r:r + 1, :], elnU4.bitcast(F32)[r:r + 1, :],
                                     st[e]["p_u0"][r:r + 1, :])
                _lrelu(nc, uo2[r:r + 1, :], uo2[r:r + 1, :])
            uo2v = uo2[:].rearrange("(a b) d -> a b d", b=64)
            nc.sync.dma_start(out[0:2, 0:1, :], uo2v[0:2, 32:33, :])

            # ============ pass A2g: grams -> msq -> var ====================
            for e in range(B_LOC):
                S = st[e]
                uat = S["uat"]
                ua2t = S["ua2t"]
                p_mu = pmue2.tile([128, 2 * N], F32, tag="mue2")
                p_e2 = pmue2.tile([128, 2 * N], F32, tag="mue2")
                for blk in range(2):
                    cs = slice(blk * 128, (blk + 1) * 128)
                    ns = slice(blk * N, (blk + 1) * N)
                    nc.tensor.matmul(p_mu[:, ns], uat[:, cs], uat[:])
                    nc.tensor.matmul(p_e2[:, ns], ua2t[:, cs], ua2t[:])
                msq_big = pool.tile([128, 2 * N], F32, tag=f"msq{e}")  # scratch
                nc.scalar.activation(msq_big[:], p_mu[:], ACTF.Square, scale=1.0 / D)
                var_e = pool.tile([128, 2 * N], F32, tag=f"var{e}")
                nc.vector.scalar_tensor_tensor(var_e[:], p_e2[:], 1.0 / D,
                                               msq_big[:],
                                               op0=ALU.mult, op1=ALU.subtract)
                S["var"] = var_e

            # ============ pass A2b: softmax denom + rstd ===================
            rstd_insts = []
            for e in range(B_LOC):
                S = st[e]
                expv = S["expv"]
                rden_cols = pool.tile([128, 2], F32, tag=f"rdenc{e}")
                for blk in range(2):
                    cs = slice(blk * 128, (blk + 1) * 128)
                    p_denc = psmall.tile([128, 2], F32, tag="small")
                    nc.tensor.matmul(p_denc[:], expv[:, 0, cs], ones2[:, 0:2],
                                     start=True, stop=False)
                    nc.tensor.matmul(p_denc[:], expv[:, 1, cs], ones2[:, 0:2],
                                     start=False, stop=True)
                    nc.vector.reciprocal(rden_cols[:, blk:blk + 1], p_denc[:, 0:1])
                S["rden_cols"] = rden_cols

                # inv-sigma; deps force it after ALL exp-set ACT ops so the
                # act-table switches exactly once to the abs-rsqrt set
                rstd_e = pool.tile([128, 2 * N], F32, tag=f"rstd{e}")
                ri = nc.scalar.activation(rstd_e[:], S["var"][:], ACTF.Abs_reciprocal_sqrt)
                rstd_insts.append(ri)
                S["rstd"] = rstd_e

            for ri in rstd_insts:
                for ei in exp_insts:
                    add_dep_helper(ri.ins, ei.ins, sync=False,
                                   reason="abs-rsqrt after all exp-set ACT ops")

            # ================= pass B: attention + output =================
            for e in range(B_LOC):
                S = st[e]
                ua_both = S["ua_both"]
                expv = S["expv"]

                btTs = []
                for J in range(2):
                    ns = slice(J * N, (J + 1) * N)
                    btT = pool.tile([128, N], F32R, tag=f"btT{J}")
                    nc.vector.tensor_mul(btT[:], expv.bitcast(F32)[:, J, :],
                                         S["rstd"][:, ns])
                    btTs.append(btT)

                p_S2 = ps.tile([128, 2, D], F32, tag="S")
                for blk in range(2):
                    cs = slice(blk * 128, (blk + 1) * 128)
                    nc.tensor.matmul(p_S2[:, blk, :], btTs[0][:, cs], ua_both[:, 0, :],
                                     start=True, stop=False)
                    nc.tensor.matmul(p_S2[:, blk, :], btTs[1][:, cs], ua_both[:, 1, :],
                                     start=False, stop=True)

                # t1b = ua*S with fused row-sum -> c
                t1b = pool.tile([128, 2, D], F32, tag="t1b")
                c_raw = pool.tile([128, 2], F32, tag="c_raw")
                for blk in range(2):
                    nc.vector.scalar_tensor_tensor(t1b[:, blk, :],
                                                   ua_both.bitcast(F32)[:, blk, :], 1.0,
                                                   p_S2[:, blk, :],
                                                   op0=ALU.mult, op1=ALU.mult,
                                                   accum_out=c_raw[:, blk:blk + 1])
                c_col = pool.tile([128, 2], F32, tag="c_col")
                nc.vector.tensor_scalar_mul(c_col[:], c_raw[:], 1.0 / D)

                o_big = pool.tile([128, 2, D], F32, tag="o_big")
                for blk in range(2):
                    rg = pool.tile([128, D], F32, tag="rg")
                    nc.vector.tensor_scalar_mul(rg[:], g_bc, S["rden_cols"][:, blk:blk + 1])
                    t2 = pool.tile([128, D], F32, tag="t2")
                    nc.vector.scalar_tensor_tensor(t2[:], t1b[:, blk, :], c_col[:, blk:blk + 1],
                                                   rg[:], op0=ALU.subtract, op1=ALU.mult)
                    t3 = pool.tile([128, D], F32, tag="t3")
                    nc.vector.tensor_add(t3[:], t2[:], b_bc)
                    _lrelu(nc, o_big[:, blk, :], t3[:])
                out_rows = out[e, 1:257, :].rearrange("(p n) d -> p n d", n=2)
                nc.sync.dma_start(out_rows, o_big[:])

    nc.compile()
    return nc


def _host_consts(Wa, ba, a_w, a_b):
    aq, ak, ai = a_w[:D], a_w[D:2 * D], a_w[2 * D:]
    vq = aq @ Wa
    vk = ak @ Wa
    vi = ai @ Wa
    c0 = float(ba @ aq + ba @ ak + ba @ ai + a_b[0])
    cstT = np.stack([vq, vk, vi], axis=1).astype(np.float32)
    cstR = np.zeros((1, 4 * D), np.float32)
    cstR[0, 2 * D] = c0
    return cstT, cstR


_NC_CACHE = {}


def _get_nc():
    if "nc" not in _NC_CACHE:
        _NC_CACHE["nc"] = build()
    return _NC_CACHE["nc"]


def run(embeddings, Wa, ba, a_w, a_b, ln_g, ln_b, **spmd_kwargs):
    embeddings = np.ascontiguousarray(embeddings, dtype=np.float32)
    cstT, cstR = _host_consts(np.asarray(Wa, np.float32), np.asarray(ba, np.float32),
                              np.asarray(a_w, np.float32), np.asarray(a_b, np.float32))
    cstR[0, 0:D] = np.asarray(ln_g, np.float32)
    cstR[0, D:2 * D] = np.asarray(ln_b, np.float32)

    nc = _get_nc()
    in_maps = [
        {"emb": embeddings[c * B_LOC:(c + 1) * B_LOC], "cstT": cstT, "cstR": cstR}
        for c in range(N_CORES)
    ]
    res = run_bass_kernel_spmd(nc, in_maps, core_ids=list(range(N_CORES)), **spmd_kwargs)
    outp = np.concatenate([res.results[c]["out"] for c in range(N_CORES)], axis=0)
    return outp, res


def kernel(embeddings, Wa, ba, a_w, a_b, ln_g, ln_b):
    outp, _ = run(embeddings, Wa, ba, a_w, a_b, ln_g, ln_b)
    return outp


# revision 44
# speedup vs baseline: 1.0295x; 1.0295x over previous
"""GAT attention kernel for Trainium2 (Bass/Tile), 8-core data parallel.

Per-core math (2 examples each, N=256 items, D=64):
  e   = LayerNorm(emb);  ua = e[0] * e[2:]
  qk  = LeakyReLU(s_q_i + s_k_j + c);  alpha = softmax_j
  attention over value_ij = LN(ua_i * ua_j) collapsed via gram matrices:
    mu = UA@UA^T/D,  E2 = UA^2@UA^2^T/D,  var = E2/D - (mu/D)^2
    rstd = exp(-0.5*ln(var + eps))   <- ln+exp live in ONE act-table set,
                                        so the kernel needs a single
                                        ACT_TABLE_LOAD (no Abs_rsqrt set)
    att_i = g*((ua_i*St_i - ct_i) * rden_i) + b
  with St = beta~@UA, beta~ = exp(qk)*rstd (unnormalized), rden = 1/sum_j exp,
  ct_i = rowsum(ua_i*St_i)/D.
  out = LeakyReLU(concat([e0*e1], att))

Perf structure vs the previous version:
  - no fp32->fp32r cast copies: fp32r tiles are written through .bitcast(F32)
    views (bit-identical), so the Scalar queue has no preamble copy chain.
  - LN statistics: square+sum fused into one DVE scalar_tensor_tensor with
    accum_out (no ACT round trip), all 5 inv-sigma values (U rows + 4 item
    blocks) batched into ONE quake-rsqrt chain.
  - scores: s_i computed with a transpose of eln(U) + a [1,128] matmul
    (kills both SBUF->SBUF iid DMAs); u0*iid output row computed in place
    at partitions {32, 96} and stored with one 2-descriptor DMA.
  - variance path: mu^2 on ACT (Square, scale), E2/D - msq fused in one DVE
    stt straight out of PSUM; rstd = exp(-0.5 ln(var+eps)) on ACT.
  - all j,i-transposed beta~ logic as before: no PE transposes of beta.
  - inputs arrive in 3 DMAs spread over 3 queues, outputs in 3 DMAs.
"""

import numpy as np

import concourse.bass as bass
from concourse import bacc
import concourse.mybir as mybir
import concourse.tile as tile
from concourse import masks
from concourse.bass_utils import run_bass_kernel_spmd
from concourse.tile import add_dep_helper

F32 = mybir.dt.float32
F32R = mybir.dt.float32r
I32 = mybir.dt.int32
ALU = mybir.AluOpType
ACTF = mybir.ActivationFunctionType
AX = mybir.AxisListType

B, NODE, D = 16, 258, 64
N = NODE - 2
N_CORES = 8
B_LOC = B // N_CORES
EPS = 1e-5
SLOPE = 0.01
OUT_ROWS = N + 1
MAGIC = 0x5f375a86


def _rsqrt(nc, pool, x, P, W, pfx):
    """x**-0.5 on DVE: bit trick + 1 Newton iteration. rel err ~1.8e-3."""
    y0 = pool.tile([P, W], F32, tag=pfx + "_y0")
    nc.vector.tensor_scalar(y0.bitcast(I32)[:], x.bitcast(I32)[:], 1, None,
                            op0=ALU.logical_shift_right)
    nc.vector.tensor_scalar(y0.bitcast(I32)[:], y0.bitcast(I32)[:], -1, MAGIC,
                            op0=ALU.mult, op1=ALU.add)
    t = pool.tile([P, W], F32, tag=pfx + "_t")
    nc.vector.tensor_mul(t[:], y0[:], y0[:])
    u = pool.tile([P, W], F32, tag=pfx + "_u")
    nc.vector.scalar_tensor_tensor(u[:], t[:], 0.5, x[:], op0=ALU.mult, op1=ALU.mult)
    v = pool.tile([P, W], F32, tag=pfx + "_v")
    nc.vector.tensor_mul(v[:], u[:], y0[:])
    r = pool.tile([P, W], F32, tag=pfx + "_r")
    nc.vector.scalar_tensor_tensor(r[:], y0[:], 1.5, v[:], op0=ALU.mult, op1=ALU.subtract)
    return r


def _lrelu(nc, out_ap, in_ap):
    nc.vector.scalar_tensor_tensor(out_ap, in_ap, SLOPE, in_ap, op0=ALU.mult, op1=ALU.max)


def build():
    nc = bacc.Bacc()
    emb = nc.dram_tensor("emb", [B_LOC, NODE, D], F32, kind="ExternalInput")
    cstT = nc.dram_tensor("cstT", [D, 3], F32R, kind="ExternalInput")   # cols: vq, vk, vi
    cstR = nc.dram_tensor("cstR", [1, 4 * D], F32R, kind="ExternalInput")  # [g|b|c0|pad]
    out = nc.dram_tensor("out", [B_LOC, OUT_ROWS, D], F32, kind="ExternalOutput")

    with tile.TileContext(nc) as tc:
        with (
            tc.tile_pool(name="const", bufs=1) as cpool,
            tc.tile_pool(name="work", bufs=2) as pool,
            tc.tile_pool(name="psmall", bufs=2, space="PSUM") as psmall,
            tc.tile_pool(name="pt", bufs=1, space="PSUM") as ppt,
            tc.tile_pool(name="pqk", bufs=1, space="PSUM") as pqk,
            tc.tile_pool(name="pmue2", bufs=2, space="PSUM") as pmue2,
            tc.tile_pool(name="ps", bufs=2, space="PSUM") as ps,
        ):
            # ---- input DMAs first on each queue: they gate the pipeline ----
            # item rows 2..257 as [128, 2, 64], row r = 2p + n
            tAB0 = pool.tile([128, 2, D], F32, tag="tAB0")
            nc.sync.dma_start(tAB0[:], emb[0, 2:258, :].rearrange("(p n) d -> p n d", n=2))
            # U+iid rows at quadrant partitions {0,32,64,96} (engine ops need
            # base partitions at quadrant boundaries)
            tU4 = cpool.tile([128, D], F32)
            u4v = tU4[:].rearrange("(a b) d -> a b d", b=32)
            nc.sync.dma_start(u4v[0:2, 0:1, :], emb[0, 0:2, :])
            nc.gpsimd.dma_start(u4v[2:4, 0:1, :], emb[1, 0:2, :])
            tAB1 = pool.tile([128, 2, D], F32, tag="tAB1")
            nc.scalar.dma_start(tAB1[:], emb[1, 2:258, :].rearrange("(p n) d -> p n d", n=2))
            in_tiles = [tAB0, tAB1]

            cst_sb = cpool.tile([1, 4 * D], F32R)
            nc.gpsimd.dma_start(cst_sb[:], cstR[:, :])
            c0_ap = cst_sb.bitcast(F32)[:, 2 * D:2 * D + 1]
            vqk = cpool.tile([D, 3], F32R)
            nc.gpsimd.dma_start(vqk[:], cstT[:, :])

            # ---- constants (casts on DVE so the ACT queue stays clean) ----
            ones2f = cpool.tile([128, 128], F32)
            nc.gpsimd.memset(ones2f[:], 1.0)
            ones2 = cpool.tile([128, 128], F32R)
            nc.vector.tensor_copy(ones2[:], ones2f[:])
            identF = cpool.tile([128, 128], F32)
            masks.make_identity(nc, identF[:])
            identR = cpool.tile([128, 128], F32R)
            nc.vector.tensor_copy(identR[:], identF[:])

            p_gb = psmall.tile([128, 2 * D], F32, tag="small")
            nc.tensor.matmul(p_gb[:], ones2[0:1, :], cst_sb[:, 0:2 * D])
            gb_bc = cpool.tile([128, 2 * D], F32)
            nc.vector.tensor_copy(gb_bc[:], p_gb[:])
            g_bc = gb_bc[:, 0:D]
            b_bc = gb_bc[:, D:2 * D]

            # ---- LN statistics, all on DVE (fused square+accum) ----
            sum5 = pool.tile([128, 5], F32, tag="sum5")
            nc.vector.reduce_sum(sum5[:, 0:1], tU4[:], axis=AX.X)
            for e in range(B_LOC):
                nc.vector.reduce_sum(sum5[:, 1 + 2 * e:3 + 2 * e], in_tiles[e][:], axis=AX.X)
            nm5 = pool.tile([128, 5], F32, tag="nm5")
            nc.vector.tensor_scalar_mul(nm5[:], sum5[:], -1.0 / D)

            ss5 = pool.tile([128, 5], F32, tag="ss5")
            xcU = pool.tile([128, D], F32, tag="xcU")
            nc.vector.tensor_scalar_add(xcU[:], tU4[:], nm5[:, 0:1])
            sq_scr = pool.tile([128, D], F32, tag="sq_scr")
            nc.vector.scalar_tensor_tensor(sq_scr[:], xcU[:], 1.0, xcU[:],
                                           op0=ALU.mult, op1=ALU.mult,
                                           accum_out=ss5[:, 0:1])
            xcs = []
            for e in range(B_LOC):
                for n in range(2):
                    xc = pool.tile([128, D], F32, tag=f"xc{e}{n}")
                    nc.vector.tensor_scalar_add(xc[:], in_tiles[e][:, n, :],
                                                nm5[:, 1 + 2 * e + n:2 + 2 * e + n])
                    scr = pool.tile([128, D], F32, tag="sq_scr2")
                    nc.vector.scalar_tensor_tensor(scr[:], xc[:], 1.0, xc[:],
                                                   op0=ALU.mult, op1=ALU.mult,
                                                   accum_out=ss5[:, 1 + 2 * e + n:2 + 2 * e + n])
                    xcs.append(xc)
            var5 = pool.tile([128, 5], F32, tag="var5")
            nc.vector.tensor_scalar(var5[:], ss5[:], 1.0 / D, EPS, op0=ALU.mult, op1=ALU.add)
            rstd5 = _rsqrt(nc, pool, var5, 128, 5, "ln5")

            # eln of U rows (all 4 quadrant rows batched)
            elnU4 = cpool.tile([128, D], F32R)
            nc.vector.scalar_tensor_tensor(elnU4[:], xcU[:], rstd5[:, 0:1],
                                           g_bc, op0=ALU.mult, op1=ALU.mult)
            nc.vector.tensor_add(elnU4[:], elnU4.bitcast(F32)[:], b_bc)

            # s_i = vi . eln(iid) for both examples via one transpose + matmul
            p_eT = psmall.tile([D, 128], F32R, tag="small")
            nc.tensor.transpose(p_eT[:], elnU4[:], identR[:])
            eT = pool.tile([D, 128], F32R, tag="eT")
            nc.vector.tensor_copy(eT[:], p_eT.bitcast(F32)[:])
            p_si = psmall.tile([1, 128], F32, tag="small")
            nc.tensor.matmul(p_si[:], vqk[:, 2:3], eT[:])
            c_all = pool.tile([1, 2], F32, tag="c_all")
            for e in range(B_LOC):
                nc.vector.tensor_scalar_add(c_all[:, e:e + 1],
                                            p_si[0:1, 32 + 64 * e:33 + 64 * e], c0_ap)

            st = [dict() for _ in range(B_LOC)]

            # ================= pass A: per-example ua + transposes =========
            uo2 = cpool.tile([128, D], F32)
            for e in range(B_LOC):
                S = st[e]
                # broadcast u0 row (same base partition trick: ones row at 64e)
                p_u0 = ps.tile([128, D], F32, tag="S")
                nc.tensor.matmul(p_u0[:], ones2[64 * e:64 * e + 1, 0:128],
                                 elnU4[64 * e:64 * e + 1, :])

                elnA = pool.tile([128, D], F32, tag="elnA")
                nc.vector.scalar_tensor_tensor(elnA[:], xcs[2 * e][:], rstd5[:, 1 + 2 * e:2 + 2 * e],
                                               g_bc, op0=ALU.mult, op1=ALU.mult)
                nc.vector.tensor_add(elnA[:], elnA[:], b_bc)
                elnB = pool.tile([128, D], F32, tag="elnB")
                nc.vector.scalar_tensor_tensor(elnB[:], xcs[2 * e + 1][:], rstd5[:, 2 + 2 * e:3 + 2 * e],
                                               g_bc, op0=ALU.mult, op1=ALU.mult)
                nc.vector.tensor_add(elnB[:], elnB[:], b_bc)
                ua_both = pool.tile([128, 2, D], F32R, tag="ua_both")
                nc.vector.tensor_mul(ua_both[:, 0, :], elnA[:], p_u0[:])
                nc.vector.tensor_mul(ua_both[:, 1, :], elnB[:], p_u0[:])
                S["ua_both"] = ua_both
                S["p_u0"] = p_u0

                # UA^T and (UA^2)^T
                p_t = ppt.tile([D, N], F32R, tag="pt")
                nc.tensor.transpose(p_t[:, 0:128], ua_both[:, 0, :], identR[:])
                nc.tensor.transpose(p_t[:, 128:256], ua_both[:, 1, :], identR[:])
                uat = pool.tile([D, N], F32R, tag="uat")
                nc.vector.tensor_copy(uat[:], p_t.bitcast(F32)[:])
                ua2t = pool.tile([D, N], F32R, tag="ua2t")
                nc.vector.tensor_mul(ua2t[:], p_t.bitcast(F32)[:],
                                     uat.bitcast(F32)[:])
                S["uat"] = uat
                S["ua2t"] = ua2t

            # ============ pass A2a: scores + exp (ACT stream) ==============
            exp_insts = []
            qk_mm_insts = []
            for e in range(B_LOC):
                S = st[e]
                uat = S["uat"]

                # s_k columns per j-block
                sk_sb = pool.tile([128, 2], F32, tag="sk_sb")
                for J in range(2):
                    cs = slice(J * 128, (J + 1) * 128)
                    p_sqk = psmall.tile([128, 2], F32, tag="small")
                    nc.tensor.matmul(p_sqk[:], uat[:, cs], vqk[:, 0:2])
                    nc.vector.tensor_copy(sk_sb[:, J:J + 1], p_sqk[:, 1:2])

                # s_q as a row, + (s_i + c0)
                p_sqrow = psmall.tile([1, N], F32, tag="small")
                nc.tensor.matmul(p_sqrow[:], vqk[:, 0:1], uat[:])
                sqc = pool.tile([1, N], F32R, tag="sqc")
                nc.scalar.activation(sqc[:], p_sqrow[:], ACTF.Identity,
                                     bias=c_all[:, e:e + 1])

                # qk^T = Prelu(bcast(s_q row) + s_k col bias); exp (unnormalized)
                p_qkT = pqk.tile([128, N], F32, tag="qk")
                qi = nc.tensor.matmul(p_qkT[:], ones2[0:1, :], sqc[:])
                qk_mm_insts.append(qi)
                expv = pool.tile([128, 2, N], F32R, tag=f"expv{e}")
                for J in range(2):
                    qkT = pool.tile([128, N], F32, tag="qkT")
                    nc.scalar.activation(qkT[:], p_qkT[:], ACTF.Prelu,
                                         bias=sk_sb[:, J:J + 1], alpha=SLOPE)
                    ei = nc.scalar.activation(expv[:, J, :], qkT[:], ACTF.Exp)
                    exp_insts.append(ei)
                S["expv"] = expv

            # u0*iid output rows, in place at partitions {32, 96}; emitted
            # this late so the tiny ops sit behind the critical DVE work
            for e in range(B_LOC):
                r = 32 + 64 * e
                nc.vector.tensor_mul(uo2[r:r + 1, :], elnU4.bitcast(F32)[r:r + 1, :],
                                     st[e]["p_u0"][r:r + 1, :])
                _lrelu(nc, uo2[r:r + 1, :], uo2[r:r + 1, :])
            uo2v = uo2[:].rearrange("(a b) d -> a b d", b=64)
            nc.sync.dma_start(out[0:2, 0:1, :], uo2v[0:2, 32:33, :])

            # ============ pass A2g: grams -> msq -> var ====================
            for e in range(B_LOC):
                S = st[e]
                uat = S["uat"]
                ua2t = S["ua2t"]
                p_mu = pmue2.tile([128, 2 * N], F32, tag="mue2")
                p_e2 = pmue2.tile([128, 2 * N], F32, tag="mue2")
                for blk in range(2):
                    cs = slice(blk * 128, (blk + 1) * 128)
                    ns = slice(blk * N, (blk + 1) * N)
                    mi = nc.tensor.matmul(p_mu[:, ns], uat[:, cs], uat[:])
                    ei2 = nc.tensor.matmul(p_e2[:, ns], ua2t[:, cs], ua2t[:])
                    # keep the PE queue prioritized on the score path: grams
                    # only after both examples' qk broadcasts are in flight
                    for qi in qk_mm_insts:
                        add_dep_helper(mi.ins, qi.ins, sync=False,
                                       reason="grams after score-path matmuls")
                        add_dep_helper(ei2.ins, qi.ins, sync=False,
                                       reason="grams after score-path matmuls")
                msq_big = pool.tile([128, 2 * N], F32, tag=f"msq{e}")  # scratch
                nc.scalar.activation(msq_big[:], p_mu[:], ACTF.Square, scale=1.0 / D)
                var_e = pool.tile([128, 2 * N], F32, tag=f"var{e}")
                nc.vector.scalar_tensor_tensor(var_e[:], p_e2[:], 1.0 / D,
                                               msq_big[:],
                                               op0=ALU.mult, op1=ALU.subtract)
                S["var"] = var_e

            # ============ pass A2b: softmax denom + rstd ===================
            rstd_insts = []
            for e in range(B_LOC):
                S = st[e]
                expv = S["expv"]
                rden_cols = pool.tile([128, 2], F32, tag=f"rdenc{e}")
                for blk in range(2):
                    cs = slice(blk * 128, (blk + 1) * 128)
                    p_denc = psmall.tile([128, 2], F32, tag="small")
                    nc.tensor.matmul(p_denc[:], expv[:, 0, cs], ones2[:, 0:2],
                                     start=True, stop=False)
                    nc.tensor.matmul(p_denc[:], expv[:, 1, cs], ones2[:, 0:2],
                                     start=False, stop=True)
                    nc.vector.reciprocal(rden_cols[:, blk:blk + 1], p_denc[:, 0:1])
                S["rden_cols"] = rden_cols

                # inv-sigma; deps force it after ALL exp-set ACT ops so the
                # act-table switches exactly once to the abs-rsqrt set
                rstd_e = pool.tile([128, 2 * N], F32, tag=f"rstd{e}")
                ri = nc.scalar.activation(rstd_e[:], S["var"][:], ACTF.Abs_reciprocal_sqrt)
                rstd_insts.append(ri)
                S["rstd"] = rstd_e

            for ri in rstd_insts:
                for ei in exp_insts:
                    add_dep_helper(ri.ins, ei.ins, sync=False,
                                   reason="abs-rsqrt after all exp-set ACT ops")

            # ================= pass B: attention + output =================
            for e in range(B_LOC):
                S = st[e]
                ua_both = S["ua_both"]
                expv = S["expv"]

                btTs = []
                for J in range(2):
                    ns = slice(J * N, (J + 1) * N)
                    btT = pool.tile([128, N], F32R, tag=f"btT{J}")
                    nc.vector.tensor_mul(btT[:], expv.bitcast(F32)[:, J, :],
                                         S["rstd"][:, ns])
                    btTs.append(btT)

                p_S2 = ps.tile([128, 2, D], F32, tag="S")
                for blk in range(2):
                    cs = slice(blk * 128, (blk + 1) * 128)
                    nc.tensor.matmul(p_S2[:, blk, :], btTs[0][:, cs], ua_both[:, 0, :],
                                     start=True, stop=False)
                    nc.tensor.matmul(p_S2[:, blk, :], btTs[1][:, cs], ua_both[:, 1, :],
                                     start=False, stop=True)

                # t1b = ua*S with fused row-sum -> c
                t1b = pool.tile([128, 2, D], F32, tag="t1b")
                c_raw = pool.tile([128, 2], F32, tag="c_raw")
                for blk in range(2):
                    nc.vector.scalar_tensor_tensor(t1b[:, blk, :],
                                                   ua_both.bitcast(F32)[:, blk, :], 1.0,
                                                   p_S2[:, blk, :],
                                                   op0=ALU.mult, op1=ALU.mult,
                                                   accum_out=c_raw[:, blk:blk + 1])
                c_col = pool.tile([128, 2], F32, tag="c_col")
                nc.vector.tensor_scalar_mul(c_col[:], c_raw[:], 1.0 / D)

                o_big = pool.tile([128, 2, D], F32, tag="o_big")
                for blk in range(2):
                    rg = pool.tile([128, D], F32, tag="rg")
                    nc.vector.tensor_scalar_mul(rg[:], g_bc, S["rden_cols"][:, blk:blk + 1])
                    t2 = pool.tile([128, D], F32, tag="t2")
                    nc.vector.scalar_tensor_tensor(t2[:], t1b[:, blk, :], c_col[:, blk:blk + 1],
                                                   rg[:], op0=ALU.subtract, op1=ALU.mult)
                    t3 = pool.tile([128, D], F32, tag="t3")
                    nc.vector.tensor_add(t3[:], t2[:], b_bc)
                    _lrelu(nc, o_big[:, blk, :], t3[:])
                out_rows = out[e, 1:257, :].rearrange("(p n) d -> p n d", n=2)
                nc.sync.dma_start(out_rows, o_big[:])

    nc.compile()
    return nc


def _host_consts(Wa, ba, a_w, a_b):
    aq, ak, ai = a_w[:D], a_w[D:2 * D], a_w[2 * D:]
    vq = aq @ Wa
    vk = ak @ Wa
    vi = ai @ Wa
    c0 = float(ba @ aq + ba @ ak + ba @ ai + a_b[0])
    cstT = np.stack([vq, vk, vi], axis=1).astype(np.float32)
    cstR = np.zeros((1, 4 * D), np.float32)
    cstR[0, 2 * D] = c0
    return cstT, cstR


_NC_CACHE = {}


def _get_nc():
    if "nc" not in _NC_CACHE:
        _NC_CACHE["nc"] = build()
    return _NC_CACHE["nc"]


def run(embeddings, Wa, ba, a_w, a_b, ln_g, ln_b, **spmd_kwargs):
    embeddings = np.ascontiguousarray(embeddings, dtype=np.float32)
    cstT, cstR = _host_consts(np.asarray(Wa, np.float32), np.asarray(ba, np.float32),
                              np.asarray(a_w, np.float32), np.asarray(a_b, np.float32))
    cstR[0, 0:D] = np.asarray(ln_g, np.float32)
    cstR[0, D:2 * D] = np.asarray(ln_b, np.float32)

    nc = _get_nc()
    in_maps = [
        {"emb": embeddings[c * B_LOC:(c + 1) * B_LOC], "cstT": cstT, "cstR": cstR}
        for c in range(N_CORES)
    ]
    res = run_bass_kernel_spmd(nc, in_maps, core_ids=list(range(N_CORES)), **spmd_kwargs)
    outp = np.concatenate([res.results[c]["out"] for c in range(N_CORES)], axis=0)
    return outp, res


def kernel(embeddings, Wa, ba, a_w, a_b, ln_g, ln_b):
    outp, _ = run(embeddings, Wa, ba, a_w, a_b, ln_g, ln_b)
    return outp


# revision 50
# speedup vs baseline: 1.0534x; 1.0233x over previous
"""GAT attention kernel for Trainium2 (Bass/Tile), 8-core data parallel.

Per-core math (2 examples each, N=256 items, D=64):
  e   = LayerNorm(emb);  ua = e[0] * e[2:]
  qk  = LeakyReLU(s_q_i + s_k_j + c);  alpha = softmax_j
  attention over value_ij = LN(ua_i * ua_j) collapsed via gram matrices:
    mu = UA@UA^T/D,  E2 = UA^2@UA^2^T/D,  var = E2/D - (mu/D)^2
    rstd = exp(-0.5*ln(var + eps))   <- ln+exp live in ONE act-table set,
                                        so the kernel needs a single
                                        ACT_TABLE_LOAD (no Abs_rsqrt set)
    att_i = g*((ua_i*St_i - ct_i) * rden_i) + b
  with St = beta~@UA, beta~ = exp(qk)*rstd (unnormalized), rden = 1/sum_j exp,
  ct_i = rowsum(ua_i*St_i)/D.
  out = LeakyReLU(concat([e0*e1], att))

Perf structure vs the previous version:
  - no fp32->fp32r cast copies: fp32r tiles are written through .bitcast(F32)
    views (bit-identical), so the Scalar queue has no preamble copy chain.
  - LN statistics: square+sum fused into one DVE scalar_tensor_tensor with
    accum_out (no ACT round trip), all 5 inv-sigma values (U rows + 4 item
    blocks) batched into ONE quake-rsqrt chain.
  - scores: s_i computed with a transpose of eln(U) + a [1,128] matmul
    (kills both SBUF->SBUF iid DMAs); u0*iid output row computed in place
    at partitions {32, 96} and stored with one 2-descriptor DMA.
  - variance path: mu^2 on ACT (Square, scale), E2/D - msq fused in one DVE
    stt straight out of PSUM; rstd = exp(-0.5 ln(var+eps)) on ACT.
  - all j,i-transposed beta~ logic as before: no PE transposes of beta.
  - inputs arrive in 3 DMAs spread over 3 queues, outputs in 3 DMAs.
"""

import numpy as np

import concourse.bass as bass
from concourse import bacc
import concourse.mybir as mybir
import concourse.tile as tile
from concourse import masks
from concourse.bass_utils import run_bass_kernel_spmd
from concourse.tile import add_dep_helper

F32 = mybir.dt.float32
F32R = mybir.dt.float32r
I32 = mybir.dt.int32
ALU = mybir.AluOpType
ACTF = mybir.ActivationFunctionType
AX = mybir.AxisListType

B, NODE, D = 16, 258, 64
N = NODE - 2
N_CORES = 8
B_LOC = B // N_CORES
EPS = 1e-5
SLOPE = 0.01
OUT_ROWS = N + 1
MAGIC = 0x5f375a86


def _rsqrt(nc, pool, x, P, W, pfx):
    """x**-0.5 on DVE: bit trick + 1 Newton iteration. rel err ~1.8e-3."""
    y0 = pool.tile([P, W], F32, tag=pfx + "_y0")
    nc.vector.tensor_scalar(y0.bitcast(I32)[:], x.bitcast(I32)[:], 1, None,
                            op0=ALU.logical_shift_right)
    nc.vector.tensor_scalar(y0.bitcast(I32)[:], y0.bitcast(I32)[:], -1, MAGIC,
                            op0=ALU.mult, op1=ALU.add)
    t = pool.tile([P, W], F32, tag=pfx + "_t")
    nc.vector.tensor_mul(t[:], y0[:], y0[:])
    u = pool.tile([P, W], F32, tag=pfx + "_u")
    nc.vector.scalar_tensor_tensor(u[:], t[:], 0.5, x[:], op0=ALU.mult, op1=ALU.mult)
    v = pool.tile([P, W], F32, tag=pfx + "_v")
    nc.vector.tensor_mul(v[:], u[:], y0[:])
    r = pool.tile([P, W], F32, tag=pfx + "_r")
    nc.vector.scalar_tensor_tensor(r[:], y0[:], 1.5, v[:], op0=ALU.mult, op1=ALU.subtract)
    return r


def _lrelu(nc, out_ap, in_ap):
    nc.vector.scalar_tensor_tensor(out_ap, in_ap, SLOPE, in_ap, op0=ALU.mult, op1=ALU.max)


def build():
    nc = bacc.Bacc()
    emb = nc.dram_tensor("emb", [B_LOC, NODE, D], F32, kind="ExternalInput")
    cstT = nc.dram_tensor("cstT", [D, 3], F32R, kind="ExternalInput")   # cols: vq, vk, vi
    cstR = nc.dram_tensor("cstR", [1, 4 * D], F32R, kind="ExternalInput")  # [g|b|c0|pad]
    out = nc.dram_tensor("out", [B_LOC, OUT_ROWS, D], F32, kind="ExternalOutput")

    with tile.TileContext(nc) as tc:
        with (
            tc.tile_pool(name="const", bufs=1) as cpool,
            tc.tile_pool(name="work", bufs=2) as pool,
            tc.tile_pool(name="psmall", bufs=2, space="PSUM") as psmall,
            tc.tile_pool(name="pt", bufs=1, space="PSUM") as ppt,
            tc.tile_pool(name="pqk", bufs=1, space="PSUM") as pqk,
            tc.tile_pool(name="pmue2", bufs=2, space="PSUM") as pmue2,
            tc.tile_pool(name="ps", bufs=2, space="PSUM") as ps,
        ):
            # ---- input DMAs first on each queue: they gate the pipeline ----
            # item rows 2..257 as [128, 2, 64], row r = 2p + n
            tAB0 = pool.tile([128, 2, D], F32, tag="tAB0")
            nc.sync.dma_start(tAB0[:], emb[0, 2:258, :].rearrange("(p n) d -> p n d", n=2))
            # U+iid rows at quadrant partitions {0,32,64,96} (engine ops need
            # base partitions at quadrant boundaries)
            tU4 = cpool.tile([128, D], F32)
            u4v = tU4[:].rearrange("(a b) d -> a b d", b=32)
            nc.sync.dma_start(u4v[0:2, 0:1, :], emb[0, 0:2, :])
            nc.gpsimd.dma_start(u4v[2:4, 0:1, :], emb[1, 0:2, :])
            tAB1 = pool.tile([128, 2, D], F32, tag="tAB1")
            nc.scalar.dma_start(tAB1[:], emb[1, 2:258, :].rearrange("(p n) d -> p n d", n=2))
            in_tiles = [tAB0, tAB1]

            cst_sb = cpool.tile([1, 4 * D], F32R)
            nc.gpsimd.dma_start(cst_sb[:], cstR[:, :])
            c0_ap = cst_sb.bitcast(F32)[:, 2 * D:2 * D + 1]
            vqk = cpool.tile([D, 3], F32R)
            nc.gpsimd.dma_start(vqk[:], cstT[:, :])

            # ---- constants (casts on DVE so the ACT queue stays clean) ----
            ones2f = cpool.tile([128, 128], F32)
            nc.gpsimd.memset(ones2f[:], 1.0)
            ones2 = cpool.tile([128, 128], F32R)
            nc.vector.tensor_copy(ones2[:], ones2f[:])
            identF = cpool.tile([128, 128], F32)
            masks.make_identity(nc, identF[:])
            identR = cpool.tile([128, 128], F32R)
            nc.vector.tensor_copy(identR[:], identF[:])

            p_gb = psmall.tile([128, 2 * D], F32, tag="small")
            nc.tensor.matmul(p_gb[:], ones2[0:1, :], cst_sb[:, 0:2 * D])
            gb_bc = cpool.tile([128, 2 * D], F32)
            nc.vector.tensor_copy(gb_bc[:], p_gb[:])
            g_bc = gb_bc[:, 0:D]
            b_bc = gb_bc[:, D:2 * D]

            # ---- LN statistics, all on DVE (fused square+accum) ----
            sum5 = pool.tile([128, 5], F32, tag="sum5")
            nc.vector.reduce_sum(sum5[:, 0:1], tU4[:], axis=AX.X)
            for e in range(B_LOC):
                nc.vector.reduce_sum(sum5[:, 1 + 2 * e:3 + 2 * e], in_tiles[e][:], axis=AX.X)
            nm5 = pool.tile([128, 5], F32, tag="nm5")
            nc.vector.tensor_scalar_mul(nm5[:], sum5[:], -1.0 / D)

            ss5 = pool.tile([128, 5], F32, tag="ss5")
            xcU = pool.tile([128, D], F32, tag="xcU")
            nc.vector.tensor_scalar_add(xcU[:], tU4[:], nm5[:, 0:1])
            sq_scr = pool.tile([128, D], F32, tag="sq_scr")
            nc.vector.scalar_tensor_tensor(sq_scr[:], xcU[:], 1.0, xcU[:],
                                           op0=ALU.mult, op1=ALU.mult,
                                           accum_out=ss5[:, 0:1])
            xcs = []
            for e in range(B_LOC):
                for n in range(2):
                    xc = pool.tile([128, D], F32, tag=f"xc{e}{n}")
                    nc.vector.tensor_scalar_add(xc[:], in_tiles[e][:, n, :],
                                                nm5[:, 1 + 2 * e + n:2 + 2 * e + n])
                    scr = pool.tile([128, D], F32, tag="sq_scr2")
                    nc.vector.scalar_tensor_tensor(scr[:], xc[:], 1.0, xc[:],
                                                   op0=ALU.mult, op1=ALU.mult,
                                                   accum_out=ss5[:, 1 + 2 * e + n:2 + 2 * e + n])
                    xcs.append(xc)
            var5 = pool.tile([128, 5], F32, tag="var5")
            nc.vector.tensor_scalar(var5[:], ss5[:], 1.0 / D, EPS, op0=ALU.mult, op1=ALU.add)
            rstd5 = _rsqrt(nc, pool, var5, 128, 5, "ln5")

            # eln of U rows (all 4 quadrant rows batched)
            elnU4 = cpool.tile([128, D], F32R)
            nc.vector.scalar_tensor_tensor(elnU4[:], xcU[:], rstd5[:, 0:1],
                                           g_bc, op0=ALU.mult, op1=ALU.mult)
            nc.vector.tensor_add(elnU4[:], elnU4.bitcast(F32)[:], b_bc)

            # s_i = vi . eln(iid) for both examples via one transpose + matmul
            p_eT = psmall.tile([D, 128], F32R, tag="small")
            nc.tensor.transpose(p_eT[:], elnU4[:], identR[:])
            eT = pool.tile([D, 128], F32R, tag="eT")
            nc.vector.tensor_copy(eT[:], p_eT.bitcast(F32)[:])
            p_si = psmall.tile([1, 128], F32, tag="small")
            nc.tensor.matmul(p_si[:], vqk[:, 2:3], eT[:])
            c_all = pool.tile([1, 2], F32, tag="c_all")
            for e in range(B_LOC):
                nc.vector.tensor_scalar_add(c_all[:, e:e + 1],
                                            p_si[0:1, 32 + 64 * e:33 + 64 * e], c0_ap)

            st = [dict() for _ in range(B_LOC)]

            # ================= pass A: per-example ua + transposes =========
            for e in range(B_LOC):
                S = st[e]
                # broadcast u0 row (same base partition trick: ones row at 64e)
                p_u0 = ps.tile([128, D], F32, tag="S")
                nc.tensor.matmul(p_u0[:], ones2[64 * e:64 * e + 1, 0:128],
                                 elnU4[64 * e:64 * e + 1, :])

                elnA = pool.tile([128, D], F32, tag="elnA")
                nc.vector.scalar_tensor_tensor(elnA[:], xcs[2 * e][:], rstd5[:, 1 + 2 * e:2 + 2 * e],
                                               g_bc, op0=ALU.mult, op1=ALU.mult)
                nc.vector.tensor_add(elnA[:], elnA[:], b_bc)
                elnB = pool.tile([128, D], F32, tag="elnB")
                nc.vector.scalar_tensor_tensor(elnB[:], xcs[2 * e + 1][:], rstd5[:, 2 + 2 * e:3 + 2 * e],
                                               g_bc, op0=ALU.mult, op1=ALU.mult)
                nc.vector.tensor_add(elnB[:], elnB[:], b_bc)
                ua_both = pool.tile([128, 2, D], F32R, tag="ua_both")
                nc.vector.tensor_mul(ua_both[:, 0, :], elnA[:], p_u0[:])
                nc.vector.tensor_mul(ua_both[:, 1, :], elnB[:], p_u0[:])
                S["ua_both"] = ua_both
                S["p_u0"] = p_u0

                # UA^T and (UA^2)^T
                p_t = ppt.tile([D, N], F32R, tag="pt")
                nc.tensor.transpose(p_t[:, 0:128], ua_both[:, 0, :], identR[:])
                nc.tensor.transpose(p_t[:, 128:256], ua_both[:, 1, :], identR[:])
                uat = pool.tile([D, N], F32R, tag="uat")
                nc.vector.tensor_copy(uat[:], p_t.bitcast(F32)[:])
                ua2t = pool.tile([D, N], F32R, tag="ua2t")
                nc.vector.tensor_mul(ua2t[:], p_t.bitcast(F32)[:],
                                     uat.bitcast(F32)[:])
                S["uat"] = uat
                S["ua2t"] = ua2t

            # ============ pass A2a: scores + exp (ACT stream) ==============
            exp_insts = []
            qk_mm_insts = []
            for e in range(B_LOC):
                S = st[e]
                uat = S["uat"]

                # s_k columns per j-block
                sk_sb = pool.tile([128, 2], F32, tag="sk_sb")
                for J in range(2):
                    cs = slice(J * 128, (J + 1) * 128)
                    p_sqk = psmall.tile([128, 2], F32, tag="small")
                    nc.tensor.matmul(p_sqk[:], uat[:, cs], vqk[:, 0:2])
                    nc.vector.tensor_copy(sk_sb[:, J:J + 1], p_sqk[:, 1:2])

                # s_q as a row, + (s_i + c0)
                p_sqrow = psmall.tile([1, N], F32, tag="small")
                nc.tensor.matmul(p_sqrow[:], vqk[:, 0:1], uat[:])
                sqc = pool.tile([1, N], F32R, tag="sqc")
                nc.scalar.activation(sqc[:], p_sqrow[:], ACTF.Identity,
                                     bias=c_all[:, e:e + 1])

                # qk^T = Prelu(bcast(s_q row) + s_k col bias); exp (unnormalized)
                if e == 0:
                    p_qk2 = pqk.tile([128, 2, N], F32, tag="qk")
                qi = nc.tensor.matmul(p_qk2[:, e, :], ones2[0:1, :], sqc[:])
                qk_mm_insts.append(qi)
                expv = pool.tile([128, 2, N], F32R, tag=f"expv{e}")
                for J in range(2):
                    qkT = pool.tile([128, N], F32, tag="qkT")
                    nc.scalar.activation(qkT[:], p_qk2[:, e, :], ACTF.Prelu,
                                         bias=sk_sb[:, J:J + 1], alpha=SLOPE)
                    ei = nc.scalar.activation(expv[:, J, :], qkT[:], ACTF.Exp)
                    exp_insts.append(ei)
                S["expv"] = expv

            # u0*iid output rows from eT columns (u0 at cols {0,64}, iid at
            # {32,96} -- same partitions), on the idle GpSimd engine; then a
            # tiny PE transpose turns the [64,2] columns into output rows
            uoC = pool.tile([D, 2], F32, tag="uoC")
            eTf = eT.bitcast(F32)
            nc.gpsimd.tensor_mul(uoC[:], eTf[:, 0:128:64], eTf[:, 32:128:64])
            p_uoT = psmall.tile([2, D], F32, tag="small")
            nc.tensor.transpose(p_uoT[:], uoC[:], identF[0:D, 0:D])
            uoR = pool.tile([2, D], F32, tag="uoR")
            nc.vector.tensor_copy(uoR[:], p_uoT[:])
            _lrelu(nc, uoR[:], uoR[:])
            nc.sync.dma_start(out[0:2, 0:1, :], uoR[:])

            # ============ pass A2g: grams -> msq -> var ====================
            for e in range(B_LOC):
                S = st[e]
                uat = S["uat"]
                ua2t = S["ua2t"]
                p_mu = pmue2.tile([128, 2 * N], F32, tag="mue2")
                p_e2 = pmue2.tile([128, 2 * N], F32, tag="mue2")
                for blk in range(2):
                    cs = slice(blk * 128, (blk + 1) * 128)
                    ns = slice(blk * N, (blk + 1) * N)
                    mi = nc.tensor.matmul(p_mu[:, ns], uat[:, cs], uat[:])
                    ei2 = nc.tensor.matmul(p_e2[:, ns], ua2t[:, cs], ua2t[:])
                    # keep the PE queue prioritized on the score path: grams
                    # only after both examples' qk broadcasts are in flight
                    for qi in qk_mm_insts:
                        add_dep_helper(mi.ins, qi.ins, sync=False,
                                       reason="grams after score-path matmuls")
                        add_dep_helper(ei2.ins, qi.ins, sync=False,
                                       reason="grams after score-path matmuls")
                msq_big = pool.tile([128, 2 * N], F32, tag=f"msq{e}")  # scratch
                nc.scalar.activation(msq_big[:], p_mu[:], ACTF.Square, scale=1.0 / D)
                var_e = pool.tile([128, 2 * N], F32, tag=f"var{e}")
                nc.vector.scalar_tensor_tensor(var_e[:], p_e2[:], 1.0 / D,
                                               msq_big[:],
                                               op0=ALU.mult, op1=ALU.subtract)
                S["var"] = var_e

            # ============ pass A2b: softmax denom + rstd ===================
            rstd_insts = []
            for e in range(B_LOC):
                S = st[e]
                expv = S["expv"]
                rden_cols = pool.tile([128, 2], F32, tag=f"rdenc{e}")
                for blk in range(2):
                    cs = slice(blk * 128, (blk + 1) * 128)
                    p_denc = psmall.tile([128, 2], F32, tag="small")
                    nc.tensor.matmul(p_denc[:], expv[:, 0, cs], ones2[:, 0:2],
                                     start=True, stop=False)
                    nc.tensor.matmul(p_denc[:], expv[:, 1, cs], ones2[:, 0:2],
                                     start=False, stop=True)
                    nc.vector.reciprocal(rden_cols[:, blk:blk + 1], p_denc[:, 0:1])
                S["rden_cols"] = rden_cols

                # inv-sigma; deps force it after ALL exp-set ACT ops so the
                # act-table switches exactly once to the abs-rsqrt set
                rstd_e = pool.tile([128, 2 * N], F32, tag=f"rstd{e}")
                ri = nc.scalar.activation(rstd_e[:], S["var"][:], ACTF.Abs_reciprocal_sqrt)
                rstd_insts.append(ri)
                S["rstd"] = rstd_e

            for ri in rstd_insts:
                for ei in exp_insts:
                    add_dep_helper(ri.ins, ei.ins, sync=False,
                                   reason="abs-rsqrt after all exp-set ACT ops")

            # ================= pass B: attention + output =================
            for e in range(B_LOC):
                S = st[e]
                ua_both = S["ua_both"]
                expv = S["expv"]

                btTs = []
                for J in range(2):
                    ns = slice(J * N, (J + 1) * N)
                    btT = pool.tile([128, N], F32R, tag=f"btT{J}")
                    nc.vector.tensor_mul(btT[:], expv.bitcast(F32)[:, J, :],
                                         S["rstd"][:, ns])
                    btTs.append(btT)

                p_S2 = ps.tile([128, 2, D], F32, tag="S")
                for blk in range(2):
                    cs = slice(blk * 128, (blk + 1) * 128)
                    nc.tensor.matmul(p_S2[:, blk, :], btTs[0][:, cs], ua_both[:, 0, :],
                                     start=True, stop=False)
                    nc.tensor.matmul(p_S2[:, blk, :], btTs[1][:, cs], ua_both[:, 1, :],
                                     start=False, stop=True)

                # t1b = ua*S with fused row-sum -> c
                t1b = pool.tile([128, 2, D], F32, tag="t1b")
                c_raw = pool.tile([128, 2], F32, tag="c_raw")
                for blk in range(2):
                    nc.vector.scalar_tensor_tensor(t1b[:, blk, :],
                                                   ua_both.bitcast(F32)[:, blk, :], 1.0,
                                                   p_S2[:, blk, :],
                                                   op0=ALU.mult, op1=ALU.mult,
                                                   accum_out=c_raw[:, blk:blk + 1])
                c_col = pool.tile([128, 2], F32, tag="c_col")
                nc.vector.tensor_scalar_mul(c_col[:], c_raw[:], 1.0 / D)

                o_big = pool.tile([128, 2, D], F32, tag="o_big")
                for blk in range(2):
                    rg = pool.tile([128, D], F32, tag="rg")
                    nc.vector.tensor_scalar_mul(rg[:], g_bc, S["rden_cols"][:, blk:blk + 1])
                    t2 = pool.tile([128, D], F32, tag="t2")
                    nc.vector.scalar_tensor_tensor(t2[:], t1b[:, blk, :], c_col[:, blk:blk + 1],
                                                   rg[:], op0=ALU.subtract, op1=ALU.mult)
                    t3 = pool.tile([128, D], F32, tag="t3")
                    nc.vector.tensor_add(t3[:], t2[:], b_bc)
                    _lrelu(nc, o_big[:, blk, :], t3[:])
                out_rows = out[e, 1:257, :].rearrange("(p n) d -> p n d", n=2)
                nc.sync.dma_start(out_rows, o_big[:])

    nc.compile()
    return nc


def _host_consts(Wa, ba, a_w, a_b):
    aq, ak, ai = a_w[:D], a_w[D:2 * D], a_w[2 * D:]
    vq = aq @ Wa
    vk = ak @ Wa
    vi = ai @ Wa
    c0 = float(ba @ aq + ba @ ak + ba @ ai + a_b[0])
    cstT = np.stack([vq, vk, vi], axis=1).astype(np.float32)
    cstR = np.zeros((1, 4 * D), np.float32)
    cstR[0, 2 * D] = c0
    return cstT, cstR


_NC_CACHE = {}


def _get_nc():
    if "nc" not in _NC_CACHE:
        _NC_CACHE["nc"] = build()
    return _NC_CACHE["nc"]


def run(embeddings, Wa, ba, a_w, a_b, ln_g, ln_b, **spmd_kwargs):
    embeddings = np.ascontiguousarray(embeddings, dtype=np.float32)
    cstT, cstR = _host_consts(np.asarray(Wa, np.float32), np.asarray(ba, np.float32),
                              np.asarray(a_w, np.float32), np.asarray(a_b, np.float32))
    cstR[0, 0:D] = np.asarray(ln_g, np.float32)
    cstR[0, D:2 * D] = np.asarray(ln_b, np.float32)

    nc = _get_nc()
    in_maps = [
        {"emb": embeddings[c * B_LOC:(c + 1) * B_LOC], "cstT": cstT, "cstR": cstR}
        for c in range(N_CORES)
    ]
    res = run_bass_kernel_spmd(nc, in_maps, core_ids=list(range(N_CORES)), **spmd_kwargs)
    outp = np.concatenate([res.results[c]["out"] for c in range(N_CORES)], axis=0)
    return outp, res


def kernel(embeddings, Wa, ba, a_w, a_b, ln_g, ln_b):
    outp, _ = run(embeddings, Wa, ba, a_w, a_b, ln_g, ln_b)
    return outp
